# revision 2
# baseline (speedup 1.0000x reference)
"""Trainium2 Bass kernel for nn_InterferenceDecoder.

out[s, v] = |sum_e conj(psi)[s,e] * patterns[v,e]|^2 + (psi_real @ W.T)[s, v] + b[v]

Strategy (8 NeuronCores, tensor-parallel on vocab):
  - vocab 50257 padded to 51200 = 8 * 6400; core i owns vocab slab [i*6400, (i+1)*6400)
  - psi replicated; patterns/W/b sharded on vocab
  - host pre-transposes operands so the contraction dim E=128 is the SBUF
    partition dim (standard weight-layout prep):
        patT_r/patT_i/wT : [128, 6400]  (= shard.T)
        psiT_r/psiT_i    : [128, 2048]  (= psi.T), psiT_in = -psi_imag.T
        b_rs             : [128, 50]    (b[v] at [v % 128, v // 128])
  - device computes the TRANSPOSED output slab out_t[v, s] ([6400, 2048]):
      per [128v x 512s] tile:
        psum_r = patR.psiR + patI.psiI        (2 matmuls, K=128 each)
        psum_i = patI.psiR + patR.(-psiI)     (2 matmuls)
        psum_l = W.psiR                       (1 matmul)
        s12    = Square(psum_ri)              (ACT, fp16 out, FD=1024)
        t      = s12[:, :512] + s12[:, 512:]  (DVE fp16 2x)
        out    = (psum_l + b_v) + t           (fused scalar_tensor_tensor)
  - host unshards: full[s, v] = out_t[v - off, s] (transpose + concat), then
    slices off the vocab padding.
"""

import sys

for _p in ("/opt/trn_rl_repo", "/opt/pypackages"):
    if _p not in sys.path:
        sys.path.append(_p)

import numpy as np

import concourse.bass as bass
import concourse.mybir as mybir
from concourse import bacc
from concourse.tile import TileContext
from concourse.bass_utils import run_bass_kernel_spmd


def _install_ntff_hook_shim():
    """Provide antenv.axon_hooks if the image lacks it, so trace=True can
    capture NTFF profiles through the axon PJRT .so."""
    try:
        from antenv import axon_hooks  # noqa: F401
        return
    except ImportError:
        pass
    import contextlib
    import ctypes
    import types

    import antenv

    so_path = "/opt/axon/libaxon_pjrt.so"
    mod = types.ModuleType("antenv.axon_hooks")
    _state = {"hook": None}

    def set_axon_ntff_profile_hook(hook):
        _state["hook"] = hook

    def get_axon_ntff_profile_hook():
        return _state["hook"]

    mod.set_axon_ntff_profile_hook = set_axon_ntff_profile_hook
    mod.get_axon_ntff_profile_hook = get_axon_ntff_profile_hook
    sys.modules["antenv.axon_hooks"] = mod
    antenv.axon_hooks = mod

    try:
        lib = ctypes.CDLL(so_path)
    except OSError:
        return
    if not hasattr(lib, "axon_start_nrt_profile"):
        return
    lib.axon_start_nrt_profile.argtypes = [
        ctypes.POINTER(ctypes.c_int64), ctypes.c_size_t]
    lib.axon_start_nrt_profile.restype = ctypes.c_int64
    lib.axon_stop_nrt_profile.argtypes = [ctypes.c_char_p]
    lib.axon_stop_nrt_profile.restype = ctypes.c_int64

    @contextlib.contextmanager
    def _hook(output_dir, device_ids):
        import jax
        jax.devices()
        if device_ids:
            ids = (ctypes.c_int64 * len(device_ids))(*device_ids)
            rc = lib.axon_start_nrt_profile(ids, len(device_ids))
        else:
            rc = lib.axon_start_nrt_profile(None, 0)
        if rc != 0:
            raise RuntimeError(f"axon_start_nrt_profile rc={rc}")
        try:
            yield
        finally:
            n = lib.axon_stop_nrt_profile(str(output_dir).encode())
            print(f"ntff profile: {n} file(s) written to {output_dir}",
                  file=sys.stderr)

    set_axon_ntff_profile_hook(_hook)


_install_ntff_hook_shim()

SEQ = 2048
EMBED = 128
VOCAB = 50257
N_CORES = 8
V_PAD = 51200            # 8 * 6400
V_CORE = V_PAD // N_CORES  # 6400
V_TILES = V_CORE // 128    # 50
S_TILE = 512
S_TILES = SEQ // S_TILE    # 4

F32 = mybir.dt.float32
F16 = mybir.dt.float16

_compiled = {}


def _build_program():
    nc = bacc.Bacc()

    pat_r = nc.dram_tensor("pat_r", [EMBED, V_CORE], F32, kind="ExternalInput")
    pat_i = nc.dram_tensor("pat_i", [EMBED, V_CORE], F32, kind="ExternalInput")
    w_t = nc.dram_tensor("w_t", [EMBED, V_CORE], F32, kind="ExternalInput")
    psi_r = nc.dram_tensor("psi_r", [EMBED, SEQ], F32, kind="ExternalInput")
    psi_i = nc.dram_tensor("psi_i", [EMBED, SEQ], F32, kind="ExternalInput")
    psi_in = nc.dram_tensor("psi_in", [EMBED, SEQ], F32, kind="ExternalInput")
    b_rs = nc.dram_tensor("b_rs", [128, V_TILES], F32, kind="ExternalInput")
    out_t = nc.dram_tensor("out_t", [V_CORE, SEQ], F32, kind="ExternalOutput")

    add = mybir.AluOpType.add

    with TileContext(nc) as tc:
        with tc.tile_pool(name="weights", bufs=1) as wpool, \
             tc.tile_pool(name="stage", bufs=3) as stpool, \
             tc.tile_pool(name="eltw", bufs=3) as epool, \
             tc.tile_pool(name="psum", bufs=2, space="PSUM") as pspool:
            patr_sb = wpool.tile([EMBED, V_CORE], F32)
            pati_sb = wpool.tile([EMBED, V_CORE], F32)
            wt_sb = wpool.tile([EMBED, V_CORE], F32)
            psir_sb = wpool.tile([EMBED, SEQ], F32)
            psii_sb = wpool.tile([EMBED, SEQ], F32)
            psiin_sb = wpool.tile([EMBED, SEQ], F32)
            b_sb = wpool.tile([128, V_TILES], F32)
            nc.sync.dma_start(out=patr_sb[:], in_=pat_r[:])
            nc.sync.dma_start(out=pati_sb[:], in_=pat_i[:])
            nc.sync.dma_start(out=wt_sb[:], in_=w_t[:])
            nc.sync.dma_start(out=psir_sb[:], in_=psi_r[:])
            nc.sync.dma_start(out=psii_sb[:], in_=psi_i[:])
            nc.sync.dma_start(out=psiin_sb[:], in_=psi_in[:])
            nc.sync.dma_start(out=b_sb[:], in_=b_rs[:])

            for v in range(V_TILES):
                vs = slice(v * 128, (v + 1) * 128)
                stage = stpool.tile([128, SEQ], F32, tag="stage")
                for s in range(S_TILES):
                    ss = slice(s * S_TILE, (s + 1) * S_TILE)
                    psum_ri = pspool.tile([128, 2 * S_TILE], F32, tag="ri")
                    psum_l = pspool.tile([128, S_TILE], F32, tag="lin")
                    nc.tensor.matmul(psum_ri[:, 0:S_TILE], patr_sb[:, vs],
                                     psir_sb[:, ss], start=True, stop=False)
                    nc.tensor.matmul(psum_ri[:, 0:S_TILE], pati_sb[:, vs],
                                     psii_sb[:, ss], start=False, stop=True)
                    nc.tensor.matmul(psum_ri[:, S_TILE:], pati_sb[:, vs],
                                     psir_sb[:, ss], start=True, stop=False)
                    nc.tensor.matmul(psum_ri[:, S_TILE:], patr_sb[:, vs],
                                     psiin_sb[:, ss], start=False, stop=True)
                    nc.tensor.matmul(psum_l[:], wt_sb[:, vs],
                                     psir_sb[:, ss], start=True, stop=True)
                    s12 = epool.tile([128, 2 * S_TILE], F16, tag="sq")
                    nc.scalar.square(s12[:], psum_ri[:])
                    t = epool.tile([128, S_TILE], F16, tag="t")
                    nc.vector.tensor_add(out=t[:], in0=s12[:, 0:S_TILE],
                                         in1=s12[:, S_TILE:])
                    nc.vector.scalar_tensor_tensor(
                        out=stage[:, ss], in0=psum_l[:],
                        scalar=b_sb[:, v:v + 1], in1=t[:],
                        op0=add, op1=add)
                nc.sync.dma_start(out=out_t[vs, :], in_=stage[:])

    nc.finalize()
    return nc


def _get_program():
    if "nc" not in _compiled:
        _compiled["nc"] = _build_program()
    return _compiled["nc"]


def kernel(psi_real, psi_imag, patterns_real, patterns_imag, W, b):
    psi_real = np.ascontiguousarray(psi_real, dtype=np.float32)
    psi_imag = np.ascontiguousarray(psi_imag, dtype=np.float32)

    psiT_r = np.ascontiguousarray(psi_real.T)
    psiT_i = np.ascontiguousarray(psi_imag.T)
    psiT_in = np.ascontiguousarray(-psi_imag.T)

    # pad vocab and pre-transpose the vocab-sharded operands
    def pad_t(m):
        full = np.zeros((EMBED, V_PAD), dtype=np.float32)
        full[:, :VOCAB] = np.asarray(m, dtype=np.float32).T
        return full

    patT_r = pad_t(patterns_real)
    patT_i = pad_t(patterns_imag)
    wT = pad_t(W)
    b_pad = np.zeros((V_PAD,), dtype=np.float32)
    b_pad[:VOCAB] = np.asarray(b, dtype=np.float32)

    in_maps = []
    for c in range(N_CORES):
        vs = slice(c * V_CORE, (c + 1) * V_CORE)
        b_shard = b_pad[vs]
        in_maps.append({
            "pat_r": np.ascontiguousarray(patT_r[:, vs]),
            "pat_i": np.ascontiguousarray(patT_i[:, vs]),
            "w_t": np.ascontiguousarray(wT[:, vs]),
            "psi_r": psiT_r,
            "psi_i": psiT_i,
            "psi_in": psiT_in,
            "b_rs": np.ascontiguousarray(b_shard.reshape(V_TILES, 128).T),
        })

    nc = _get_program()
    res = run_bass_kernel_spmd(nc, in_maps, core_ids=list(range(N_CORES)))
    kernel.last_results = res

    out = np.empty((SEQ, V_PAD), dtype=np.float32)
    for c in range(N_CORES):
        out[:, c * V_CORE:(c + 1) * V_CORE] = res.results[c]["out_t"].T
    return out[:, :VOCAB]


# revision 3
# speedup vs baseline: 3.1769x; 3.1769x over previous
"""Trainium2 Bass kernel for nn_InterferenceDecoder.

out[s, v] = |sum_e conj(psi)[s,e] * patterns[v,e]|^2 + (psi_real @ W.T)[s, v] + b[v]

Strategy (8 NeuronCores, tensor-parallel on vocab):
  - vocab 50257 padded to 51200 = 8 * 6400; core i owns vocab slab [i*6400, (i+1)*6400)
  - psi replicated; patterns/W/b sharded on vocab
  - host pre-transposes operands so the contraction dim E=128 is the SBUF
    partition dim (standard weight-layout prep):
        patT_r/patT_i/wT : [128, 6400]  (= shard.T)
        psiT_r/psiT_i    : [128, 2048]  (= psi.T), psiT_in = -psi_imag.T
        b_rs             : [128, 50]    (b[v] at [v % 128, v // 128])
  - device computes the TRANSPOSED output slab out_t[v, s] ([6400, 2048]):
      per [128v x 512s] tile:
        psum_r = patR.psiR + patI.psiI        (2 matmuls, K=128 each)
        psum_i = patI.psiR + patR.(-psiI)     (2 matmuls)
        psum_l = W.psiR                       (1 matmul)
        s12    = Square(psum_ri)              (ACT, fp16 out, FD=1024)
        t      = s12[:, :512] + s12[:, 512:]  (DVE fp16 2x)
        out    = (psum_l + b_v) + t           (fused scalar_tensor_tensor)
  - host unshards: full[s, v] = out_t[v - off, s] (transpose + concat), then
    slices off the vocab padding.
"""

import sys

for _p in ("/opt/trn_rl_repo", "/opt/pypackages"):
    if _p not in sys.path:
        sys.path.append(_p)

import numpy as np

import concourse.bass as bass
import concourse.mybir as mybir
from concourse import bacc
from concourse.tile import TileContext
from concourse.bass_utils import run_bass_kernel_spmd


def _install_ntff_hook_shim():
    """Provide antenv.axon_hooks if the image lacks it, so trace=True can
    capture NTFF profiles through the axon PJRT .so."""
    try:
        from antenv import axon_hooks  # noqa: F401
        return
    except ImportError:
        pass
    import contextlib
    import ctypes
    import types

    import antenv

    so_path = "/opt/axon/libaxon_pjrt.so"
    mod = types.ModuleType("antenv.axon_hooks")
    _state = {"hook": None}

    def set_axon_ntff_profile_hook(hook):
        _state["hook"] = hook

    def get_axon_ntff_profile_hook():
        return _state["hook"]

    mod.set_axon_ntff_profile_hook = set_axon_ntff_profile_hook
    mod.get_axon_ntff_profile_hook = get_axon_ntff_profile_hook
    sys.modules["antenv.axon_hooks"] = mod
    antenv.axon_hooks = mod

    try:
        lib = ctypes.CDLL(so_path)
    except OSError:
        return
    if not hasattr(lib, "axon_start_nrt_profile"):
        return
    lib.axon_start_nrt_profile.argtypes = [
        ctypes.POINTER(ctypes.c_int64), ctypes.c_size_t]
    lib.axon_start_nrt_profile.restype = ctypes.c_int64
    lib.axon_stop_nrt_profile.argtypes = [ctypes.c_char_p]
    lib.axon_stop_nrt_profile.restype = ctypes.c_int64

    @contextlib.contextmanager
    def _hook(output_dir, device_ids):
        import jax
        jax.devices()
        if device_ids:
            ids = (ctypes.c_int64 * len(device_ids))(*device_ids)
            rc = lib.axon_start_nrt_profile(ids, len(device_ids))
        else:
            rc = lib.axon_start_nrt_profile(None, 0)
        if rc != 0:
            raise RuntimeError(f"axon_start_nrt_profile rc={rc}")
        try:
            yield
        finally:
            n = lib.axon_stop_nrt_profile(str(output_dir).encode())
            print(f"ntff profile: {n} file(s) written to {output_dir}",
                  file=sys.stderr)

    set_axon_ntff_profile_hook(_hook)


_install_ntff_hook_shim()

SEQ = 2048
EMBED = 128
VOCAB = 50257
N_CORES = 8
V_PAD = 51200            # 8 * 6400
V_CORE = V_PAD // N_CORES  # 6400
V_TILES = V_CORE // 128    # 50
S_TILE = 512
S_TILES = SEQ // S_TILE    # 4

F32 = mybir.dt.float32
F16 = mybir.dt.float16

_compiled = {}


def _build_program():
    nc = bacc.Bacc()

    pat_r = nc.dram_tensor("pat_r", [EMBED, V_CORE], F16, kind="ExternalInput")
    pat_i = nc.dram_tensor("pat_i", [EMBED, V_CORE], F16, kind="ExternalInput")
    w_t = nc.dram_tensor("w_t", [EMBED, V_CORE], F16, kind="ExternalInput")
    psi_r = nc.dram_tensor("psi_r", [EMBED, SEQ], F16, kind="ExternalInput")
    psi_i = nc.dram_tensor("psi_i", [EMBED, SEQ], F16, kind="ExternalInput")
    psi_in = nc.dram_tensor("psi_in", [EMBED, SEQ], F16, kind="ExternalInput")
    b_rs = nc.dram_tensor("b_rs", [128, V_TILES], F32, kind="ExternalInput")
    out_t = nc.dram_tensor("out_t", [V_CORE, SEQ], F32, kind="ExternalOutput")

    add = mybir.AluOpType.add

    with TileContext(nc) as tc:
        with tc.tile_pool(name="weights", bufs=1) as wpool, \
             tc.tile_pool(name="stage", bufs=3) as stpool, \
             tc.tile_pool(name="eltw", bufs=3) as epool, \
             tc.tile_pool(name="psum", bufs=2, space="PSUM") as pspool:
            patr_sb = wpool.tile([EMBED, V_CORE], F16)
            pati_sb = wpool.tile([EMBED, V_CORE], F16)
            wt_sb = wpool.tile([EMBED, V_CORE], F16)
            psir_sb = wpool.tile([EMBED, SEQ], F16)
            psii_sb = wpool.tile([EMBED, SEQ], F16)
            psiin_sb = wpool.tile([EMBED, SEQ], F16)
            b_sb = wpool.tile([128, V_TILES], F32)
            nc.sync.dma_start(out=patr_sb[:], in_=pat_r[:])
            nc.sync.dma_start(out=pati_sb[:], in_=pat_i[:])
            nc.sync.dma_start(out=wt_sb[:], in_=w_t[:])
            nc.sync.dma_start(out=psir_sb[:], in_=psi_r[:])
            nc.sync.dma_start(out=psii_sb[:], in_=psi_i[:])
            nc.sync.dma_start(out=psiin_sb[:], in_=psi_in[:])
            nc.sync.dma_start(out=b_sb[:], in_=b_rs[:])

            for v in range(V_TILES):
                vs = slice(v * 128, (v + 1) * 128)
                stage = stpool.tile([128, SEQ], F32, tag="stage")
                for s in range(S_TILES):
                    ss = slice(s * S_TILE, (s + 1) * S_TILE)
                    psum_ri = pspool.tile([128, 2 * S_TILE], F32, tag="ri")
                    psum_l = pspool.tile([128, S_TILE], F32, tag="lin")
                    nc.tensor.matmul(psum_ri[:, 0:S_TILE], patr_sb[:, vs],
                                     psir_sb[:, ss], start=True, stop=False)
                    nc.tensor.matmul(psum_ri[:, 0:S_TILE], pati_sb[:, vs],
                                     psii_sb[:, ss], start=False, stop=True)
                    nc.tensor.matmul(psum_ri[:, S_TILE:], pati_sb[:, vs],
                                     psir_sb[:, ss], start=True, stop=False)
                    nc.tensor.matmul(psum_ri[:, S_TILE:], patr_sb[:, vs],
                                     psiin_sb[:, ss], start=False, stop=True)
                    nc.tensor.matmul(psum_l[:], wt_sb[:, vs],
                                     psir_sb[:, ss], start=True, stop=True)
                    s12 = epool.tile([128, 2 * S_TILE], F16, tag="sq")
                    nc.scalar.square(s12[:], psum_ri[:])
                    t = epool.tile([128, S_TILE], F16, tag="t")
                    nc.vector.tensor_add(out=t[:], in0=s12[:, 0:S_TILE],
                                         in1=s12[:, S_TILE:])
                    nc.vector.scalar_tensor_tensor(
                        out=stage[:, ss], in0=psum_l[:],
                        scalar=b_sb[:, v:v + 1], in1=t[:],
                        op0=add, op1=add)
                nc.sync.dma_start(out=out_t[vs, :], in_=stage[:])

    nc.finalize()
    return nc


def _get_program():
    if "nc" not in _compiled:
        _compiled["nc"] = _build_program()
    return _compiled["nc"]


def kernel(psi_real, psi_imag, patterns_real, patterns_imag, W, b):
    psi_real = np.ascontiguousarray(psi_real, dtype=np.float32)
    psi_imag = np.ascontiguousarray(psi_imag, dtype=np.float32)

    psiT_r = np.ascontiguousarray(psi_real.T.astype(np.float16))
    psiT_i = np.ascontiguousarray(psi_imag.T.astype(np.float16))
    psiT_in = np.ascontiguousarray((-psi_imag.T).astype(np.float16))

    # pad vocab and pre-transpose the vocab-sharded operands
    def pad_t(m):
        full = np.zeros((EMBED, V_PAD), dtype=np.float16)
        full[:, :VOCAB] = np.asarray(m, dtype=np.float32).T.astype(np.float16)
        return full

    patT_r = pad_t(patterns_real)
    patT_i = pad_t(patterns_imag)
    wT = pad_t(W)
    b_pad = np.zeros((V_PAD,), dtype=np.float32)
    b_pad[:VOCAB] = np.asarray(b, dtype=np.float32)

    in_maps = []
    for c in range(N_CORES):
        vs = slice(c * V_CORE, (c + 1) * V_CORE)
        b_shard = b_pad[vs]
        in_maps.append({
            "pat_r": np.ascontiguousarray(patT_r[:, vs]),
            "pat_i": np.ascontiguousarray(patT_i[:, vs]),
            "w_t": np.ascontiguousarray(wT[:, vs]),
            "psi_r": psiT_r,
            "psi_i": psiT_i,
            "psi_in": psiT_in,
            "b_rs": np.ascontiguousarray(b_shard.reshape(V_TILES, 128).T),
        })

    nc = _get_program()
    res = run_bass_kernel_spmd(nc, in_maps, core_ids=list(range(N_CORES)))
    kernel.last_results = res

    out = np.empty((SEQ, V_PAD), dtype=np.float32)
    for c in range(N_CORES):
        out[:, c * V_CORE:(c + 1) * V_CORE] = res.results[c]["out_t"].T
    return out[:, :VOCAB]


# revision 7
# speedup vs baseline: 3.1858x; 1.0028x over previous
"""Trainium2 Bass kernel for nn_InterferenceDecoder.

out[s, v] = |sum_e conj(psi)[s,e] * patterns[v,e]|^2 + (psi_real @ W.T)[s, v] + b[v]

Strategy (8 NeuronCores, tensor-parallel on vocab):
  - vocab 50257 padded to 51200 = 8 * 6400; core i owns vocab slab [i*6400, (i+1)*6400)
  - psi replicated; patterns/W/b sharded on vocab
  - host pre-transposes operands so the contraction dim E=128 is the SBUF
    partition dim (standard weight-layout prep):
        patT_r/patT_i/wT : [128, 6400]  (= shard.T)
        psiT_r/psiT_i    : [128, 2048]  (= psi.T), psiT_in = -psi_imag.T
        b_rs             : [128, 50]    (b[v] at [v % 128, v // 128])
  - device computes the TRANSPOSED output slab out_t[v, s] ([6400, 2048]):
      per [128v x 512s] tile:
        psum_r = patR.psiR + patI.psiI        (2 matmuls, K=128 each)
        psum_i = patI.psiR + patR.(-psiI)     (2 matmuls)
        psum_l = W.psiR                       (1 matmul)
        s12    = Square(psum_ri)              (ACT, fp16 out, FD=1024)
        t      = s12[:, :512] + s12[:, 512:]  (DVE fp16 2x)
        out    = (psum_l + b_v) + t           (fused scalar_tensor_tensor)
  - host unshards: full[s, v] = out_t[v - off, s] (transpose + concat), then
    slices off the vocab padding.
"""

import sys

for _p in ("/opt/trn_rl_repo", "/opt/pypackages"):
    if _p not in sys.path:
        sys.path.append(_p)

import numpy as np

import concourse.bass as bass
import concourse.mybir as mybir
from concourse import bacc
from concourse.tile import TileContext
from concourse.bass_utils import run_bass_kernel_spmd


def _install_ntff_hook_shim():
    """Provide antenv.axon_hooks if the image lacks it, so trace=True can
    capture NTFF profiles through the axon PJRT .so."""
    try:
        from antenv import axon_hooks  # noqa: F401
        return
    except ImportError:
        pass
    import contextlib
    import ctypes
    import types

    import antenv

    so_path = "/opt/axon/libaxon_pjrt.so"
    mod = types.ModuleType("antenv.axon_hooks")
    _state = {"hook": None}

    def set_axon_ntff_profile_hook(hook):
        _state["hook"] = hook

    def get_axon_ntff_profile_hook():
        return _state["hook"]

    mod.set_axon_ntff_profile_hook = set_axon_ntff_profile_hook
    mod.get_axon_ntff_profile_hook = get_axon_ntff_profile_hook
    sys.modules["antenv.axon_hooks"] = mod
    antenv.axon_hooks = mod

    try:
        lib = ctypes.CDLL(so_path)
    except OSError:
        return
    if not hasattr(lib, "axon_start_nrt_profile"):
        return
    lib.axon_start_nrt_profile.argtypes = [
        ctypes.POINTER(ctypes.c_int64), ctypes.c_size_t]
    lib.axon_start_nrt_profile.restype = ctypes.c_int64
    lib.axon_stop_nrt_profile.argtypes = [ctypes.c_char_p]
    lib.axon_stop_nrt_profile.restype = ctypes.c_int64

    @contextlib.contextmanager
    def _hook(output_dir, device_ids):
        import jax
        jax.devices()
        if device_ids:
            ids = (ctypes.c_int64 * len(device_ids))(*device_ids)
            rc = lib.axon_start_nrt_profile(ids, len(device_ids))
        else:
            rc = lib.axon_start_nrt_profile(None, 0)
        if rc != 0:
            raise RuntimeError(f"axon_start_nrt_profile rc={rc}")
        try:
            yield
        finally:
            n = lib.axon_stop_nrt_profile(str(output_dir).encode())
            print(f"ntff profile: {n} file(s) written to {output_dir}",
                  file=sys.stderr)

    set_axon_ntff_profile_hook(_hook)


_install_ntff_hook_shim()

SEQ = 2048
EMBED = 128
VOCAB = 50257
N_CORES = 8
V_PAD = 51200            # 8 * 6400
V_CORE = V_PAD // N_CORES  # 6400
V_TILES = V_CORE // 128    # 50
S_TILE = 512
S_TILES = SEQ // S_TILE    # 4

F32 = mybir.dt.float32
F16 = mybir.dt.float16

_compiled = {}


def _build_program():
    nc = bacc.Bacc()

    pat_r = nc.dram_tensor("pat_r", [EMBED, V_CORE], F16, kind="ExternalInput")
    pat_i = nc.dram_tensor("pat_i", [EMBED, V_CORE], F16, kind="ExternalInput")
    w_t = nc.dram_tensor("w_t", [EMBED, V_CORE], F16, kind="ExternalInput")
    psi_r = nc.dram_tensor("psi_r", [EMBED, SEQ], F16, kind="ExternalInput")
    psi_i = nc.dram_tensor("psi_i", [EMBED, SEQ], F16, kind="ExternalInput")
    psi_in = nc.dram_tensor("psi_in", [EMBED, SEQ], F16, kind="ExternalInput")
    b_rs = nc.dram_tensor("b_rs", [128, V_TILES], F32, kind="ExternalInput")
    out_t = nc.dram_tensor("out_t", [V_CORE, SEQ], F32, kind="ExternalOutput")

    add = mybir.AluOpType.add

    mult = mybir.AluOpType.mult

    with TileContext(nc) as tc:
        with tc.tile_pool(name="weights", bufs=1) as wpool, \
             tc.tile_pool(name="stage", bufs=3) as stpool, \
             tc.tile_pool(name="eltw", bufs=4) as epool, \
             tc.tile_pool(name="psum", bufs=2, space="PSUM") as pspool, \
             tc.tile_pool(name="psum_l", bufs=4, space="PSUM") as plpool:
            patr_sb = wpool.tile([EMBED, V_CORE], F16)
            pati_sb = wpool.tile([EMBED, V_CORE], F16)
            wt_sb = wpool.tile([EMBED, V_CORE], F16)
            psir_sb = wpool.tile([EMBED, SEQ], F16)
            psii_sb = wpool.tile([EMBED, SEQ], F16)
            psiin_sb = wpool.tile([EMBED, SEQ], F16)
            b_sb = wpool.tile([128, V_TILES], F32)
            nc.sync.dma_start(out=patr_sb[:], in_=pat_r[:])
            nc.sync.dma_start(out=pati_sb[:], in_=pat_i[:])
            nc.sync.dma_start(out=wt_sb[:], in_=w_t[:])
            nc.sync.dma_start(out=psir_sb[:], in_=psi_r[:])
            nc.sync.dma_start(out=psii_sb[:], in_=psi_i[:])
            nc.sync.dma_start(out=psiin_sb[:], in_=psi_in[:])
            nc.sync.dma_start(out=b_sb[:], in_=b_rs[:])

            for v in range(V_TILES):
                vs = slice(v * 128, (v + 1) * 128)
                stage = stpool.tile([128, SEQ], F32, tag="stage")
                for s in range(S_TILES):
                    ss = slice(s * S_TILE, (s + 1) * S_TILE)
                    psum_ri = pspool.tile([128, 2 * S_TILE], F32, tag="ri")
                    psum_l = plpool.tile([128, S_TILE], F32, tag="lin")
                    nc.tensor.matmul(psum_ri[:, 0:S_TILE], patr_sb[:, vs],
                                     psir_sb[:, ss], start=True, stop=False)
                    nc.tensor.matmul(psum_ri[:, 0:S_TILE], pati_sb[:, vs],
                                     psii_sb[:, ss], start=False, stop=True)
                    nc.tensor.matmul(psum_ri[:, S_TILE:], pati_sb[:, vs],
                                     psir_sb[:, ss], start=True, stop=False)
                    nc.tensor.matmul(psum_ri[:, S_TILE:], patr_sb[:, vs],
                                     psiin_sb[:, ss], start=False, stop=True)
                    nc.tensor.matmul(psum_l[:], wt_sb[:, vs],
                                     psir_sb[:, ss], start=True, stop=True)
                    s12 = epool.tile([128, 2 * S_TILE], F16, tag="sq")
                    nc.scalar.square(s12[:], psum_ri[:])
                    t = epool.tile([128, S_TILE], F16, tag="t")
                    nc.gpsimd.tensor_add(out=t[:], in0=s12[:, 0:S_TILE],
                                         in1=s12[:, S_TILE:])
                    nc.vector.scalar_tensor_tensor(
                        out=stage[:, ss], in0=psum_l[:],
                        scalar=b_sb[:, v:v + 1], in1=t[:],
                        op0=add, op1=add)
                nc.sync.dma_start(out=out_t[vs, :], in_=stage[:])

    nc.finalize()
    return nc


def _get_program():
    if "nc" not in _compiled:
        _compiled["nc"] = _build_program()
    return _compiled["nc"]


def kernel(psi_real, psi_imag, patterns_real, patterns_imag, W, b):
    psi_real = np.ascontiguousarray(psi_real, dtype=np.float32)
    psi_imag = np.ascontiguousarray(psi_imag, dtype=np.float32)

    psiT_r = np.ascontiguousarray(psi_real.T.astype(np.float16))
    psiT_i = np.ascontiguousarray(psi_imag.T.astype(np.float16))
    psiT_in = np.ascontiguousarray((-psi_imag.T).astype(np.float16))

    # pad vocab and pre-transpose the vocab-sharded operands
    def pad_t(m):
        full = np.zeros((EMBED, V_PAD), dtype=np.float16)
        full[:, :VOCAB] = np.asarray(m, dtype=np.float32).T.astype(np.float16)
        return full

    patT_r = pad_t(patterns_real)
    patT_i = pad_t(patterns_imag)
    wT = pad_t(W)
    b_pad = np.zeros((V_PAD,), dtype=np.float32)
    b_pad[:VOCAB] = np.asarray(b, dtype=np.float32)

    in_maps = []
    for c in range(N_CORES):
        vs = slice(c * V_CORE, (c + 1) * V_CORE)
        b_shard = b_pad[vs]
        in_maps.append({
            "pat_r": np.ascontiguousarray(patT_r[:, vs]),
            "pat_i": np.ascontiguousarray(patT_i[:, vs]),
            "w_t": np.ascontiguousarray(wT[:, vs]),
            "psi_r": psiT_r,
            "psi_i": psiT_i,
            "psi_in": psiT_in,
            "b_rs": np.ascontiguousarray(b_shard.reshape(V_TILES, 128).T),
        })

    nc = _get_program()
    res = run_bass_kernel_spmd(nc, in_maps, core_ids=list(range(N_CORES)))
    kernel.last_results = res

    out = np.empty((SEQ, V_PAD), dtype=np.float32)
    for c in range(N_CORES):
        out[:, c * V_CORE:(c + 1) * V_CORE] = res.results[c]["out_t"].T
    return out[:, :VOCAB]


# revision 9
# speedup vs baseline: 3.4270x; 1.0757x over previous
"""Trainium2 Bass kernel for nn_InterferenceDecoder.

out[s, v] = |sum_e conj(psi)[s,e] * patterns[v,e]|^2 + (psi_real @ W.T)[s, v] + b[v]

Strategy (8 NeuronCores, tensor-parallel on vocab):
  - vocab 50257 padded to 51200 = 8 * 6400; core i owns vocab slab [i*6400, (i+1)*6400)
  - psi replicated; patterns/W/b sharded on vocab
  - host pre-transposes operands so the contraction dim E=128 is the SBUF
    partition dim (standard weight-layout prep):
        patT_r/patT_i/wT : [128, 6400]  (= shard.T)
        psiT_r/psiT_i    : [128, 2048]  (= psi.T), psiT_in = -psi_imag.T
        b_rs             : [128, 50]    (b[v] at [v % 128, v // 128])
  - device computes the TRANSPOSED output slab out_t[v, s] ([6400, 2048]):
      per [128v x 512s] tile:
        psum_r = patR.psiR + patI.psiI        (2 matmuls, K=128 each)
        psum_i = patI.psiR + patR.(-psiI)     (2 matmuls)
        psum_l = W.psiR                       (1 matmul)
        s12    = Square(psum_ri)              (ACT, fp16 out, FD=1024)
        t      = s12[:, :512] + s12[:, 512:]  (DVE fp16 2x)
        out    = (psum_l + b_v) + t           (fused scalar_tensor_tensor)
  - host unshards: full[s, v] = out_t[v - off, s] (transpose + concat), then
    slices off the vocab padding.
"""

import sys

for _p in ("/opt/trn_rl_repo", "/opt/pypackages"):
    if _p not in sys.path:
        sys.path.append(_p)

import numpy as np

import concourse.bass as bass
import concourse.mybir as mybir
from concourse import bacc
from concourse.tile import TileContext
from concourse.bass_utils import run_bass_kernel_spmd


def _install_ntff_hook_shim():
    """Provide antenv.axon_hooks if the image lacks it, so trace=True can
    capture NTFF profiles through the axon PJRT .so."""
    try:
        from antenv import axon_hooks  # noqa: F401
        return
    except ImportError:
        pass
    import contextlib
    import ctypes
    import types

    import antenv

    so_path = "/opt/axon/libaxon_pjrt.so"
    mod = types.ModuleType("antenv.axon_hooks")
    _state = {"hook": None}

    def set_axon_ntff_profile_hook(hook):
        _state["hook"] = hook

    def get_axon_ntff_profile_hook():
        return _state["hook"]

    mod.set_axon_ntff_profile_hook = set_axon_ntff_profile_hook
    mod.get_axon_ntff_profile_hook = get_axon_ntff_profile_hook
    sys.modules["antenv.axon_hooks"] = mod
    antenv.axon_hooks = mod

    try:
        lib = ctypes.CDLL(so_path)
    except OSError:
        return
    if not hasattr(lib, "axon_start_nrt_profile"):
        return
    lib.axon_start_nrt_profile.argtypes = [
        ctypes.POINTER(ctypes.c_int64), ctypes.c_size_t]
    lib.axon_start_nrt_profile.restype = ctypes.c_int64
    lib.axon_stop_nrt_profile.argtypes = [ctypes.c_char_p]
    lib.axon_stop_nrt_profile.restype = ctypes.c_int64

    @contextlib.contextmanager
    def _hook(output_dir, device_ids):
        import jax
        jax.devices()
        if device_ids:
            ids = (ctypes.c_int64 * len(device_ids))(*device_ids)
            rc = lib.axon_start_nrt_profile(ids, len(device_ids))
        else:
            rc = lib.axon_start_nrt_profile(None, 0)
        if rc != 0:
            raise RuntimeError(f"axon_start_nrt_profile rc={rc}")
        try:
            yield
        finally:
            n = lib.axon_stop_nrt_profile(str(output_dir).encode())
            print(f"ntff profile: {n} file(s) written to {output_dir}",
                  file=sys.stderr)

    set_axon_ntff_profile_hook(_hook)


_install_ntff_hook_shim()

SEQ = 2048
EMBED = 128
VOCAB = 50257
N_CORES = 8
V_PAD = 51200            # 8 * 6400
V_CORE = V_PAD // N_CORES  # 6400
V_TILES = V_CORE // 128    # 50
S_TILE = 512
S_TILES = SEQ // S_TILE    # 4

F32 = mybir.dt.float32
F16 = mybir.dt.float16

_compiled = {}


def _build_program():
    nc = bacc.Bacc()

    pat_r = nc.dram_tensor("pat_r", [EMBED, V_CORE], F16, kind="ExternalInput")
    pat_i = nc.dram_tensor("pat_i", [EMBED, V_CORE], F16, kind="ExternalInput")
    w_t = nc.dram_tensor("w_t", [EMBED, V_CORE], F16, kind="ExternalInput")
    psi_r = nc.dram_tensor("psi_r", [EMBED, SEQ], F16, kind="ExternalInput")
    psi_i = nc.dram_tensor("psi_i", [EMBED, SEQ], F16, kind="ExternalInput")
    psi_in = nc.dram_tensor("psi_in", [EMBED, SEQ], F16, kind="ExternalInput")
    b_rs = nc.dram_tensor("b_rs", [128, V_TILES], F32, kind="ExternalInput")
    out_t = nc.dram_tensor("out_t", [V_CORE, SEQ], F32, kind="ExternalOutput")

    add = mybir.AluOpType.add

    mult = mybir.AluOpType.mult

    with TileContext(nc) as tc:
        with tc.tile_pool(name="weights", bufs=1) as wpool, \
             tc.tile_pool(name="stage", bufs=3) as stpool, \
             tc.tile_pool(name="eltw", bufs=4) as epool, \
             tc.tile_pool(name="psum", bufs=2, space="PSUM") as pspool, \
             tc.tile_pool(name="psum_l", bufs=4, space="PSUM") as plpool:
            patr_sb = wpool.tile([EMBED, V_CORE], F16)
            pati_sb = wpool.tile([EMBED, V_CORE], F16)
            wt_sb = wpool.tile([EMBED, V_CORE], F16)
            psir_sb = wpool.tile([EMBED, SEQ], F16)
            psii_sb = wpool.tile([EMBED, SEQ], F16)
            psiin_sb = wpool.tile([EMBED, SEQ], F16)
            b_sb = wpool.tile([128, V_TILES], F32)
            nc.sync.dma_start(out=psir_sb[:], in_=psi_r[:])
            nc.sync.dma_start(out=psii_sb[:], in_=psi_i[:])
            nc.sync.dma_start(out=psiin_sb[:], in_=psi_in[:])
            nc.sync.dma_start(out=b_sb[:], in_=b_rs[:])
            # chunked weight loads so the first v-tiles' matmuls start as
            # soon as their slice lands instead of after the full 10MB
            CHUNK = 10 * 128
            for c0 in range(0, V_CORE, CHUNK):
                cs = slice(c0, min(c0 + CHUNK, V_CORE))
                nc.sync.dma_start(out=patr_sb[:, cs], in_=pat_r[:, cs])
                nc.sync.dma_start(out=pati_sb[:, cs], in_=pat_i[:, cs])
                nc.sync.dma_start(out=wt_sb[:, cs], in_=w_t[:, cs])

            for v in range(V_TILES):
                vs = slice(v * 128, (v + 1) * 128)
                stage = stpool.tile([128, SEQ], F32, tag="stage")
                for s in range(S_TILES):
                    ss = slice(s * S_TILE, (s + 1) * S_TILE)
                    psum_ri = pspool.tile([128, 2 * S_TILE], F32, tag="ri")
                    psum_l = plpool.tile([128, S_TILE], F32, tag="lin")
                    nc.tensor.matmul(psum_ri[:, 0:S_TILE], patr_sb[:, vs],
                                     psir_sb[:, ss], start=True, stop=False)
                    nc.tensor.matmul(psum_ri[:, 0:S_TILE], pati_sb[:, vs],
                                     psii_sb[:, ss], start=False, stop=True)
                    nc.tensor.matmul(psum_ri[:, S_TILE:], pati_sb[:, vs],
                                     psir_sb[:, ss], start=True, stop=False)
                    nc.tensor.matmul(psum_ri[:, S_TILE:], patr_sb[:, vs],
                                     psiin_sb[:, ss], start=False, stop=True)
                    nc.tensor.matmul(psum_l[:], wt_sb[:, vs],
                                     psir_sb[:, ss], start=True, stop=True)
                    s12 = epool.tile([128, 2 * S_TILE], F16, tag="sq")
                    nc.scalar.square(s12[:], psum_ri[:])
                    t = epool.tile([128, S_TILE], F16, tag="t")
                    nc.vector.tensor_add(out=t[:], in0=s12[:, 0:S_TILE],
                                         in1=s12[:, S_TILE:])
                    nc.vector.scalar_tensor_tensor(
                        out=stage[:, ss], in0=psum_l[:],
                        scalar=b_sb[:, v:v + 1], in1=t[:],
                        op0=add, op1=add)
                nc.sync.dma_start(out=out_t[vs, :], in_=stage[:])

    nc.finalize()
    return nc


def _get_program():
    if "nc" not in _compiled:
        _compiled["nc"] = _build_program()
    return _compiled["nc"]


def kernel(psi_real, psi_imag, patterns_real, patterns_imag, W, b):
    psi_real = np.ascontiguousarray(psi_real, dtype=np.float32)
    psi_imag = np.ascontiguousarray(psi_imag, dtype=np.float32)

    psiT_r = np.ascontiguousarray(psi_real.T.astype(np.float16))
    psiT_i = np.ascontiguousarray(psi_imag.T.astype(np.float16))
    psiT_in = np.ascontiguousarray((-psi_imag.T).astype(np.float16))

    # pad vocab and pre-transpose the vocab-sharded operands
    def pad_t(m):
        full = np.zeros((EMBED, V_PAD), dtype=np.float16)
        full[:, :VOCAB] = np.asarray(m, dtype=np.float32).T.astype(np.float16)
        return full

    patT_r = pad_t(patterns_real)
    patT_i = pad_t(patterns_imag)
    wT = pad_t(W)
    b_pad = np.zeros((V_PAD,), dtype=np.float32)
    b_pad[:VOCAB] = np.asarray(b, dtype=np.float32)

    in_maps = []
    for c in range(N_CORES):
        vs = slice(c * V_CORE, (c + 1) * V_CORE)
        b_shard = b_pad[vs]
        in_maps.append({
            "pat_r": np.ascontiguousarray(patT_r[:, vs]),
            "pat_i": np.ascontiguousarray(patT_i[:, vs]),
            "w_t": np.ascontiguousarray(wT[:, vs]),
            "psi_r": psiT_r,
            "psi_i": psiT_i,
            "psi_in": psiT_in,
            "b_rs": np.ascontiguousarray(b_shard.reshape(V_TILES, 128).T),
        })

    nc = _get_program()
    res = run_bass_kernel_spmd(nc, in_maps, core_ids=list(range(N_CORES)))
    kernel.last_results = res

    out = np.empty((SEQ, V_PAD), dtype=np.float32)
    for c in range(N_CORES):
        out[:, c * V_CORE:(c + 1) * V_CORE] = res.results[c]["out_t"].T
    return out[:, :VOCAB]


# revision 12
# speedup vs baseline: 3.6042x; 1.0517x over previous
"""Trainium2 Bass kernel for nn_InterferenceDecoder.

out[s, v] = |sum_e conj(psi)[s,e] * patterns[v,e]|^2 + (psi_real @ W.T)[s, v] + b[v]

Strategy (8 NeuronCores, tensor-parallel on vocab):
  - vocab 50257 padded to 51200 = 8 * 6400; core i owns vocab slab [i*6400, (i+1)*6400)
  - psi replicated; patterns/W/b sharded on vocab
  - host pre-transposes operands so the contraction dim E=128 is the SBUF
    partition dim (standard weight-layout prep):
        patT_r/patT_i/wT : [128, 6400]  (= shard.T)
        psiT_r/psiT_i    : [128, 2048]  (= psi.T), psiT_in = -psi_imag.T
        b_rs             : [128, 50]    (b[v] at [v % 128, v // 128])
  - device computes the TRANSPOSED output slab out_t[v, s] ([6400, 2048]):
      per [128v x 512s] tile:
        psum_r = patR.psiR + patI.psiI        (2 matmuls, K=128 each)
        psum_i = patI.psiR + patR.(-psiI)     (2 matmuls)
        psum_l = W.psiR                       (1 matmul)
        s12    = Square(psum_ri)              (ACT, fp16 out, FD=1024)
        t      = s12[:, :512] + s12[:, 512:]  (DVE fp16 2x)
        out    = (psum_l + b_v) + t           (fused scalar_tensor_tensor)
  - host unshards: full[s, v] = out_t[v - off, s] (transpose + concat), then
    slices off the vocab padding.
"""

import sys

for _p in ("/opt/trn_rl_repo", "/opt/pypackages"):
    if _p not in sys.path:
        sys.path.append(_p)

import numpy as np

import concourse.bass as bass
import concourse.mybir as mybir
from concourse import bacc
from concourse.tile import TileContext
from concourse.bass_utils import run_bass_kernel_spmd


def _install_ntff_hook_shim():
    """Provide antenv.axon_hooks if the image lacks it, so trace=True can
    capture NTFF profiles through the axon PJRT .so."""
    try:
        from antenv import axon_hooks  # noqa: F401
        return
    except ImportError:
        pass
    import contextlib
    import ctypes
    import types

    import antenv

    so_path = "/opt/axon/libaxon_pjrt.so"
    mod = types.ModuleType("antenv.axon_hooks")
    _state = {"hook": None}

    def set_axon_ntff_profile_hook(hook):
        _state["hook"] = hook

    def get_axon_ntff_profile_hook():
        return _state["hook"]

    mod.set_axon_ntff_profile_hook = set_axon_ntff_profile_hook
    mod.get_axon_ntff_profile_hook = get_axon_ntff_profile_hook
    sys.modules["antenv.axon_hooks"] = mod
    antenv.axon_hooks = mod

    try:
        lib = ctypes.CDLL(so_path)
    except OSError:
        return
    if not hasattr(lib, "axon_start_nrt_profile"):
        return
    lib.axon_start_nrt_profile.argtypes = [
        ctypes.POINTER(ctypes.c_int64), ctypes.c_size_t]
    lib.axon_start_nrt_profile.restype = ctypes.c_int64
    lib.axon_stop_nrt_profile.argtypes = [ctypes.c_char_p]
    lib.axon_stop_nrt_profile.restype = ctypes.c_int64

    @contextlib.contextmanager
    def _hook(output_dir, device_ids):
        import jax
        jax.devices()
        if device_ids:
            ids = (ctypes.c_int64 * len(device_ids))(*device_ids)
            rc = lib.axon_start_nrt_profile(ids, len(device_ids))
        else:
            rc = lib.axon_start_nrt_profile(None, 0)
        if rc != 0:
            raise RuntimeError(f"axon_start_nrt_profile rc={rc}")
        try:
            yield
        finally:
            n = lib.axon_stop_nrt_profile(str(output_dir).encode())
            print(f"ntff profile: {n} file(s) written to {output_dir}",
                  file=sys.stderr)

    set_axon_ntff_profile_hook(_hook)


_install_ntff_hook_shim()

SEQ = 2048
EMBED = 128
VOCAB = 50257
N_CORES = 8
V_PAD = 51200            # 8 * 6400
V_CORE = V_PAD // N_CORES  # 6400
V_TILES = V_CORE // 128    # 50
S_TILE = 512
S_TILES = SEQ // S_TILE    # 4

F32 = mybir.dt.float32
F16 = mybir.dt.float16

_compiled = {}


def _build_program():
    nc = bacc.Bacc()

    pat_r = nc.dram_tensor("pat_r", [EMBED, V_CORE], F16, kind="ExternalInput")
    pat_i = nc.dram_tensor("pat_i", [EMBED, V_CORE], F16, kind="ExternalInput")
    w_t = nc.dram_tensor("w_t", [EMBED, V_CORE], F16, kind="ExternalInput")
    psi_r = nc.dram_tensor("psi_r", [EMBED, SEQ], F16, kind="ExternalInput")
    psi_i = nc.dram_tensor("psi_i", [EMBED, SEQ], F16, kind="ExternalInput")
    psi_in = nc.dram_tensor("psi_in", [EMBED, SEQ], F16, kind="ExternalInput")
    b_rs = nc.dram_tensor("b_rs", [128, V_TILES], F32, kind="ExternalInput")
    out_t = nc.dram_tensor("out_t", [V_CORE, SEQ], F32, kind="ExternalOutput")

    add = mybir.AluOpType.add

    mult = mybir.AluOpType.mult

    with TileContext(nc) as tc:
        with tc.tile_pool(name="weights", bufs=1) as wpool, \
             tc.tile_pool(name="stage", bufs=3) as stpool, \
             tc.tile_pool(name="eltw", bufs=4) as epool, \
             tc.tile_pool(name="psum", bufs=2, space="PSUM") as pspool, \
             tc.tile_pool(name="psum_l", bufs=2, space="PSUM") as plpool:
            patr_sb = wpool.tile([EMBED, V_CORE], F16)
            pati_sb = wpool.tile([EMBED, V_CORE], F16)
            wt_sb = wpool.tile([EMBED, V_CORE], F16)
            psir_sb = wpool.tile([EMBED, SEQ], F16)
            psii_sb = wpool.tile([EMBED, SEQ], F16)
            psiin_sb = wpool.tile([EMBED, SEQ], F16)
            b_sb = wpool.tile([128, V_TILES], F32)
            # load order: unblock the first tile's matmuls asap — psi_r +
            # a small first weight chunk, then the rest in larger chunks
            nc.sync.dma_start(out=psir_sb[:], in_=psi_r[:])
            first = slice(0, 2 * 128)
            nc.sync.dma_start(out=patr_sb[:, first], in_=pat_r[:, first])
            nc.sync.dma_start(out=psii_sb[:], in_=psi_i[:])
            nc.sync.dma_start(out=pati_sb[:, first], in_=pat_i[:, first])
            nc.sync.dma_start(out=psiin_sb[:], in_=psi_in[:])
            nc.sync.dma_start(out=wt_sb[:, first], in_=w_t[:, first])
            nc.sync.dma_start(out=b_sb[:], in_=b_rs[:])
            CHUNK = 8 * 128
            for c0 in range(2 * 128, V_CORE, CHUNK):
                cs = slice(c0, min(c0 + CHUNK, V_CORE))
                nc.sync.dma_start(out=patr_sb[:, cs], in_=pat_r[:, cs])
                nc.sync.dma_start(out=pati_sb[:, cs], in_=pat_i[:, cs])
                nc.sync.dma_start(out=wt_sb[:, cs], in_=w_t[:, cs])

            for v in range(V_TILES):
                vs = slice(v * 128, (v + 1) * 128)
                stage = stpool.tile([128, SEQ], F32, tag="stage")
                for sp in range(S_TILES // 2):
                    psum_lp = plpool.tile([128, 2 * S_TILE], F32, tag="lin")
                    t_pair = epool.tile([128, 2 * S_TILE], F16, tag="t")
                    for si in range(2):
                        s = 2 * sp + si
                        ss = slice(s * S_TILE, (s + 1) * S_TILE)
                        half = slice(si * S_TILE, (si + 1) * S_TILE)
                        psum_ri = pspool.tile([128, 2 * S_TILE], F32, tag="ri")
                        nc.tensor.matmul(psum_ri[:, 0:S_TILE], patr_sb[:, vs],
                                         psir_sb[:, ss], start=True, stop=False)
                        nc.tensor.matmul(psum_ri[:, 0:S_TILE], pati_sb[:, vs],
                                         psii_sb[:, ss], start=False, stop=True)
                        nc.tensor.matmul(psum_ri[:, S_TILE:], pati_sb[:, vs],
                                         psir_sb[:, ss], start=True, stop=False)
                        nc.tensor.matmul(psum_ri[:, S_TILE:], patr_sb[:, vs],
                                         psiin_sb[:, ss], start=False, stop=True)
                        nc.tensor.matmul(psum_lp[:, half], wt_sb[:, vs],
                                         psir_sb[:, ss], start=True, stop=True)
                        s12 = epool.tile([128, 2 * S_TILE], F16, tag="sq")
                        nc.scalar.square(s12[:], psum_ri[:])
                        nc.vector.tensor_add(out=t_pair[:, half],
                                             in0=s12[:, 0:S_TILE],
                                             in1=s12[:, S_TILE:])
                    # one fused (lin + b) + |score|^2 over the s-pair
                    pss = slice(sp * 2 * S_TILE, (sp + 1) * 2 * S_TILE)
                    nc.vector.scalar_tensor_tensor(
                        out=stage[:, pss], in0=psum_lp[:],
                        scalar=b_sb[:, v:v + 1], in1=t_pair[:],
                        op0=add, op1=add)
                nc.sync.dma_start(out=out_t[vs, :], in_=stage[:])

    nc.finalize()
    return nc


def _get_program():
    if "nc" not in _compiled:
        _compiled["nc"] = _build_program()
    return _compiled["nc"]


def kernel(psi_real, psi_imag, patterns_real, patterns_imag, W, b):
    psi_real = np.ascontiguousarray(psi_real, dtype=np.float32)
    psi_imag = np.ascontiguousarray(psi_imag, dtype=np.float32)

    psiT_r = np.ascontiguousarray(psi_real.T.astype(np.float16))
    psiT_i = np.ascontiguousarray(psi_imag.T.astype(np.float16))
    psiT_in = np.ascontiguousarray((-psi_imag.T).astype(np.float16))

    # pad vocab and pre-transpose the vocab-sharded operands
    def pad_t(m):
        full = np.zeros((EMBED, V_PAD), dtype=np.float16)
        full[:, :VOCAB] = np.asarray(m, dtype=np.float32).T.astype(np.float16)
        return full

    patT_r = pad_t(patterns_real)
    patT_i = pad_t(patterns_imag)
    wT = pad_t(W)
    b_pad = np.zeros((V_PAD,), dtype=np.float32)
    b_pad[:VOCAB] = np.asarray(b, dtype=np.float32)

    in_maps = []
    for c in range(N_CORES):
        vs = slice(c * V_CORE, (c + 1) * V_CORE)
        b_shard = b_pad[vs]
        in_maps.append({
            "pat_r": np.ascontiguousarray(patT_r[:, vs]),
            "pat_i": np.ascontiguousarray(patT_i[:, vs]),
            "w_t": np.ascontiguousarray(wT[:, vs]),
            "psi_r": psiT_r,
            "psi_i": psiT_i,
            "psi_in": psiT_in,
            "b_rs": np.ascontiguousarray(b_shard.reshape(V_TILES, 128).T),
        })

    nc = _get_program()
    res = run_bass_kernel_spmd(nc, in_maps, core_ids=list(range(N_CORES)))
    kernel.last_results = res

    out = np.empty((SEQ, V_PAD), dtype=np.float32)
    for c in range(N_CORES):
        out[:, c * V_CORE:(c + 1) * V_CORE] = res.results[c]["out_t"].T
    return out[:, :VOCAB]
